# revision 8
# baseline (speedup 1.0000x reference)
"""Single-head attention (B=8, S=2048, H=768, D=64) on 8 TRN2 NeuronCores.

Data-parallel over batch: core b computes batch element b end to end; no
collectives. Host prepacks Q/K/V into SBUF-native [128, ...] bf16 layouts
(linear DMA, 128 big descriptors per transfer) and does the final softmax
divide + transpose; the device ships unnormalized O^T plus denominators
as raw f32 PSUM DMAs.

Schedule (v3, from perfetto analysis):
  - The ACT exp stream (32 x [128,1024], ~35.6us busy) is the pacing
    resource; everything else is arranged so exp never waits.
  - Input DMA rides the SP queue in exact consumption order
    k0,q0,q1,k1,v0,k2,k3,v1,q2,q3 (quarter granules for k/q). The
    weights ride the ACT queue: wqk first (critical), ident+cst after.
    wvv is prepended to the v half-0 transfer.
  - ~55 PE warmup matmuls on a memset tile bridge the whole DMA lead-in
    so the HAM clock gate never closes before the projections.
  - Projections are single-chunk col-packed concurrent pairs; the k/q
    chunk chain (DMA sem -> proj -> DVE drain -> score) is the first-exp
    critical path: ~19us.
  - Softmax denominators: DVE sums pth tile pairs, PE does 8 4x-col-
    packed quad matmuls on the pair sums (2.8us PE instead of 8.8).
  - O^T half-0 pairs spread ~1 per period through phase B; v half-1
    projection+transpose pulled into phase-A slack (v1 lands ~29us).
"""

import os
from contextlib import ExitStack

import numpy as np
import ml_dtypes

import concourse.bass as bass
import concourse.mybir as mybir
import concourse.tile as tile
from concourse import bacc
from concourse.bass_utils import run_bass_kernel_spmd

S, H, D = 2048, 768, 64
P = 128
NT = S // P      # 16 sk tiles
HT = H // P      # 6 h tiles
CH = 512         # sq chunk = matmul free dim = PSUM bank
NCH = S // CH    # 4
BF = mybir.dt.bfloat16
F32 = mybir.dt.float32
AF = mybir.ActivationFunctionType

LAST_RESULT = None  # BassKernelResults of the most recent run (for test.py)


def _build(debug=False):
    nc = bacc.Bacc()
    qpk_d = nc.declare_dram_parameter("qpk", [P, NCH * HT * CH], BF, isOutput=False)
    kpk_d = nc.declare_dram_parameter("kpk", [P, NCH * HT * CH], BF, isOutput=False)
    # v prepack: {wvv | v half0 | v half1}
    VW = HT * 2 * CH  # 6144 elements per v half
    vpk_d = nc.declare_dram_parameter("vpk", [P, HT * P + 2 * VW], BF, isOutput=False)
    wqk_d = nc.declare_dram_parameter("wqk", [P, HT * P], BF, isOutput=False)
    ict_d = nc.declare_dram_parameter("ict", [P, P], BF, isOutput=False)
    cst_d = nc.declare_dram_parameter("cst", [P, 4 + NT], F32, isOutput=False)
    o1_d = nc.declare_dram_parameter("o1", [P, CH], BF, isOutput=True)
    o2_d = nc.declare_dram_parameter("o2", [P, CH], BF, isOutput=True)
    o3_d = nc.declare_dram_parameter("o3", [P, CH], BF, isOutput=True)

    with ExitStack() as ctx:
        tc = ctx.enter_context(tile.TileContext(nc))
        consts = ctx.enter_context(tc.tile_pool(name="consts", bufs=1))
        stages = ctx.enter_context(tc.tile_pool(name="stages", bufs=1))
        persist = ctx.enter_context(tc.tile_pool(name="persist", bufs=1))
        ppool = ctx.enter_context(tc.tile_pool(name="ppool", bufs=2 * NT))
        p2pool = ctx.enter_context(tc.tile_pool(name="p2pool", bufs=NT))
        psc = ctx.enter_context(tc.tile_pool(name="psc", bufs=1, space="PSUM"))
        psw = ctx.enter_context(tc.tile_pool(name="psw", bufs=1, space="PSUM"))
        psav = ctx.enter_context(tc.tile_pool(name="psav", bufs=1, space="PSUM"))

        # ---- constants on the scalar queue: wqk first (critical path) ----
        wqk_sb = consts.tile([P, HT * P], BF, tag="wqk")
        nc.scalar.dma_start(out=wqk_sb, in_=wqk_d[:, :])
        ict_sb = consts.tile([P, P], BF, tag="ict")
        nc.scalar.dma_start(out=ict_sb, in_=ict_d[:, :])
        cst_sb = consts.tile([P, 4 + NT], F32, tag="cst")
        nc.scalar.dma_start(out=cst_sb, in_=cst_d[:, :])
        ident_bf = ict_sb
        bvv_sb = cst_sb[:, 1:2]
        bkk_sb = cst_sb[:, 2:3]
        bqq_sb = cst_sb[:, 3:4]
        mb_sb = cst_sb[:, 4 : 4 + NT]
        warmT = consts.tile([P, P], BF, tag="warmT")
        nc.vector.memset(warmT, 0.0)
        ones32 = consts.tile([P, 32], BF, tag="ones32")
        nc.vector.memset(ones32, 1.0)

        def w_qk(h, lo, hi):  # packed q|k weight slice [128, hi-lo]
            return wqk_sb[:, h * P + lo : h * P + hi]

        # ---- input staging: linear transfers on the sync queue in exact
        # consumption order k0,q0,q1,k1,v0(+wvv),k2,k3,v1,q2,q3 ----
        stq = stages.tile([P, NCH, HT * CH], BF, tag="stq")
        stk = stages.tile([P, NCH, HT * CH], BF, tag="stk")
        stv = stages.tile([P, 2, VW], BF, tag="stv")
        wvv_sb = stages.tile([P, HT * P], BF, tag="wvv")
        QW = HT * CH  # 3072 elements per k/q quarter

        def dma_kq(dst, src, c):
            nc.sync.dma_start(out=dst[:, c, :], in_=src[:, c * QW : (c + 1) * QW])

        dma_kq(stk, kpk_d, 0)
        dma_kq(stq, qpk_d, 0)
        dma_kq(stq, qpk_d, 1)
        dma_kq(stk, kpk_d, 1)
        nc.sync.dma_start(out=wvv_sb, in_=vpk_d[:, 0 : HT * P])
        nc.sync.dma_start(out=stv[:, 0, :], in_=vpk_d[:, HT * P : HT * P + VW])
        dma_kq(stk, kpk_d, 2)
        dma_kq(stk, kpk_d, 3)
        nc.sync.dma_start(out=stv[:, 1, :], in_=vpk_d[:, HT * P + VW : HT * P + 2 * VW])
        dma_kq(stq, qpk_d, 2)
        dma_kq(stq, qpk_d, 3)

        def w_vv(h, lo, hi):
            return wvv_sb[:, h * P + lo : h * P + hi]

        def st_kq(st, h, c):  # [128, 512] h-tile slice of chunk c
            return st[:, c, h * CH : (h + 1) * CH]

        def st_v(h, half, i):  # [128, 512] h-tile slice of chunk 2*half+i
            return stv[:, half, h * 2 * CH + i * CH : h * 2 * CH + (i + 1) * CH]

        # ---- persistent SBUF tensors ----
        qqT_sb = persist.tile([P, S], BF, tag="qqT")
        kkT_sb = persist.tile([P, S], BF, tag="kkT")
        vT2_sb = persist.tile([P, S // 2], BF, tag="vT2")
        vE_sb = persist.tile([P, NT * D], BF, tag="vE")
        osb = persist.tile([P, 3 * CH], BF, tag="osb")  # output staging

        # ---- PE warmup: bridge the whole DMA lead-in ----
        warm = psw.tile([P, CH], F32, tag="aux", name="warm")
        for i in range(55):
            nc.tensor.matmul(
                warm[:, :P],
                lhsT=warmT,
                rhs=warmT,
                start=True,
                stop=True,
                skip_group_check=True,
            )

        # ---- helper blocks ----
        def kq1proj(kind, c, ptag):
            """single-chunk projection (kind 0 = k -> kkT, 1 = q -> qqT):
            one col-packed concurrent pair produces native+duplicated
            partition halves of chunk c in 6x512 cycles, then one [128,512]
            bias drain."""
            dst = kkT_sb if kind == 0 else qqT_sb
            bias = bkk_sb if kind == 0 else bqq_sb
            wlo = D if kind == 0 else 0
            stx = stk if kind == 0 else stq
            pp = psav.tile([P, CH], F32, tag=ptag, name=f"pp{kind}_{c}")
            for h in range(HT):
                nc.tensor.matmul(
                    pp[:D, :],
                    lhsT=w_qk(h, wlo, wlo + D),
                    rhs=st_kq(stx, h, c),
                    start=(h == 0),
                    stop=(h == HT - 1),
                    tile_position=(0, 0),
                    skip_group_check=True,
                )
                nc.tensor.matmul(
                    pp[D:, :],
                    lhsT=w_qk(h, wlo, wlo + D),
                    rhs=st_kq(stx, h, c),
                    start=(h == 0),
                    stop=(h == HT - 1),
                    tile_position=(0, D),
                    skip_group_check=True,
                )
            nc.vector.tensor_scalar_add(
                out=dst[:, c * CH : (c + 1) * CH], in0=pp, scalar1=bias
            )

        pth = {}

        def scores_exp(t, half):
            """scores for sk-tile t over sq half (row group alternates with
            t to hide LDWEIGHTS), exp straight into a bf16 tile."""
            lo, hi = (0, D) if t % 2 == 0 else (D, P)
            ps = psc.tile([P, 2 * CH], F32, tag=f"sc{t % 2}",
                          name=f"ps{t}_{half}")
            for sub in range(2):
                c = 2 * half + sub
                nc.tensor.matmul(
                    ps[:, sub * CH : (sub + 1) * CH],
                    lhsT=kkT_sb[lo:hi, t * P : (t + 1) * P],
                    rhs=qqT_sb[lo:hi, c * CH : (c + 1) * CH],
                    start=True,
                    stop=True,
                    tile_position=(lo, 0),
                    skip_group_check=True,
                )
            pt = ppool.tile([P, 2 * CH], BF, tag="pT", name=f"pt{t}_{half}")
            nc.scalar.activation(
                out=pt,
                in_=ps,
                func=AF.Exp,
                bias=mb_sb[:, t : t + 1],
                scale=0.125,
            )
            pth[t, half] = pt

        p2 = {}

        def pair_sum(k, half):
            """DVE: p2[k,half] = pth[2k,half] + pth[2k+1,half]."""
            t2 = p2pool.tile([P, 2 * CH], BF, tag="p2", name=f"p2_{k}_{half}")
            nc.vector.tensor_add(
                out=t2, in0=pth[2 * k, half], in1=pth[2 * k + 1, half]
            )
            p2[k, half] = t2

        pav = {}

        def vE_slice(t):
            b = (t // 8) * 4 + (t % 4)
            half = (t % 8) // 4
            return vE_sb[:, b * P + half * D : b * P + half * D + D]

        def av(t, half):
            """col-packed concurrent O^T accumulation pair for sk-tile t:
            chunk 2*half -> partitions 0:64, chunk 2*half+1 -> 64:128."""
            key = "av23" if half == 0 else "av01"
            if half not in pav:
                pav[half] = psav.tile([P, CH], F32, tag=key, name=f"pav{half}")
            vt = vE_slice(t)
            nc.tensor.matmul(
                pav[half][:D, :],
                lhsT=vt,
                rhs=pth[t, half][:, :CH],
                start=(t == 0),
                stop=(t == NT - 1),
                tile_position=(0, 0),
                skip_group_check=True,
            )
            nc.tensor.matmul(
                pav[half][D:, :],
                lhsT=vt,
                rhs=pth[t, half][:, CH:],
                start=(t == 0),
                stop=(t == NT - 1),
                tile_position=(0, D),
                skip_group_check=True,
            )

        pden = [None]

        def den4q(k):
            """4x col-packed denominator quad on pair-sum tiles: chunk ci's
            denominator accumulates in partitions 32ci:32ci+32."""
            if pden[0] is None:
                pden[0] = psw.tile([P, CH], F32, tag="den", name="pden")
            for ci in range(NCH):
                nc.tensor.matmul(
                    pden[0][32 * ci : 32 * (ci + 1), :],
                    lhsT=ones32[:, :],
                    rhs=p2[k, ci // 2][:, (ci % 2) * CH : (ci % 2 + 1) * CH],
                    start=(k == 0),
                    stop=(k == NT // 2 - 1),
                    tile_position=(0, 32 * ci),
                    skip_group_check=True,
                )

        pv_t = {}

        def v_proj_mm(u, hs=(0, HT)):
            """v projection matmuls (h range hs) + bias for chunk-pair u:
            chunk 2u -> partitions 0:64, chunk 2u+1 -> 64:128 (stacked)."""
            if u not in pv_t:
                pv_t[u] = psw.tile([P, CH], F32,
                                   tag="den" if u == 0 else "aux",
                                   name=f"pv{u}")
            pv = pv_t[u]
            for h in range(*hs):
                nc.tensor.matmul(
                    pv[:D, :],
                    lhsT=w_vv(h, 0, D),
                    rhs=st_v(h, u, 0),
                    start=(h == 0),
                    stop=(h == HT - 1),
                    tile_position=(0, 0),
                    skip_group_check=True,
                )
                nc.tensor.matmul(
                    pv[D:, :],
                    lhsT=w_vv(h, D, P),
                    rhs=st_v(h, u, 1),
                    start=(h == 0),
                    stop=(h == HT - 1),
                    tile_position=(0, D),
                    skip_group_check=True,
                )
            if hs[1] == HT:
                nc.vector.tensor_scalar_add(
                    out=vT2_sb[:, u * CH : (u + 1) * CH], in0=pv,
                    scalar1=bvv_sb,
                )

        def v_transpose(u, j):
            """one [128,128] PE-transpose block of vT2 into vE
            (block b = 4u+j holds sk tiles 8u+j and 8u+j+4)."""
            pt = psw.tile([P, P], BF, tag="den" if u == 0 else "aux",
                          name=f"ptv{u}_{j}")
            nc.tensor.transpose(
                pt,
                in_=vT2_sb[:, u * CH + j * P : u * CH + (j + 1) * P],
                identity=ident_bf,
            )
            b = 4 * u + j
            nc.vector.tensor_copy(out=vE_sb[:, b * P : (b + 1) * P], in_=pt)

        # ---- phase A schedule ----
        kq1proj(0, 0, "av01")   # k chunk 0 (arrives first)
        kq1proj(1, 0, "av23")   # q chunk 0
        kq1proj(1, 1, "av01")   # q chunk 1
        scores_exp(0, 0)
        scores_exp(1, 0)
        pair_sum(0, 0)
        scores_exp(2, 0)
        scores_exp(3, 0)
        kq1proj(0, 1, "av23")   # k chunk 1
        pair_sum(1, 0)
        scores_exp(4, 0)
        scores_exp(5, 0)
        v_proj_mm(0)            # v half 0
        pair_sum(2, 0)
        scores_exp(6, 0)
        v_transpose(0, 0)
        v_transpose(0, 1)
        scores_exp(7, 0)
        kq1proj(0, 2, "av01")   # k chunk 2
        pair_sum(3, 0)
        v_transpose(0, 2)
        v_transpose(0, 3)
        scores_exp(8, 0)
        scores_exp(9, 0)
        kq1proj(0, 3, "av23")   # k chunk 3
        pair_sum(4, 0)
        scores_exp(10, 0)
        scores_exp(11, 0)
        v_proj_mm(1, (0, 3))    # v half 1 (lands ~29us)
        pair_sum(5, 0)
        scores_exp(12, 0)
        v_proj_mm(1, (3, HT))
        scores_exp(13, 0)
        v_transpose(1, 0)
        v_transpose(1, 1)
        pair_sum(6, 0)
        scores_exp(14, 0)
        v_transpose(1, 2)
        v_transpose(1, 3)
        scores_exp(15, 0)
        kq1proj(1, 2, "av01")   # q chunk 2
        kq1proj(1, 3, "av23")   # q chunk 3
        pair_sum(7, 0)

        # ---- phase B: exp stream; av1 trails 2 tiles; av0 spread ~1 per
        # period; den quads follow the DVE pair-sums ----
        scores_exp(0, 1)
        av(0, 0)
        scores_exp(1, 1)
        av(1, 0)
        pair_sum(0, 1)
        scores_exp(2, 1)
        av(0, 1)
        av(2, 0)
        scores_exp(3, 1)
        av(1, 1)
        pair_sum(1, 1)
        den4q(0)
        av(3, 0)
        scores_exp(4, 1)
        av(2, 1)
        av(4, 0)
        scores_exp(5, 1)
        av(3, 1)
        pair_sum(2, 1)
        den4q(1)
        av(5, 0)
        scores_exp(6, 1)
        av(4, 1)
        av(6, 0)
        scores_exp(7, 1)
        av(5, 1)
        pair_sum(3, 1)
        den4q(2)
        av(7, 0)
        scores_exp(8, 1)
        av(6, 1)
        av(8, 0)
        scores_exp(9, 1)
        av(7, 1)
        pair_sum(4, 1)
        den4q(3)
        av(9, 0)
        scores_exp(10, 1)
        av(8, 1)
        av(10, 0)
        scores_exp(11, 1)
        av(9, 1)
        pair_sum(5, 1)
        den4q(4)
        av(11, 0)
        scores_exp(12, 1)
        av(10, 1)
        av(12, 0)
        av(13, 0)
        scores_exp(13, 1)
        av(11, 1)
        pair_sum(6, 1)
        den4q(5)
        av(14, 0)
        av(15, 0)
        nc.vector.tensor_copy(out=osb[:, 0:CH], in_=pav[0])
        nc.sync.dma_start(out=o1_d[:, :], in_=osb[:, 0:CH])
        scores_exp(14, 1)
        av(12, 1)
        scores_exp(15, 1)
        av(13, 1)
        pair_sum(7, 1)
        den4q(6)
        av(14, 1)
        av(15, 1)
        den4q(7)

        # ---- epilogue ----
        nc.vector.tensor_copy(out=osb[:, CH : 2 * CH], in_=pav[1])
        nc.sync.dma_start(out=o2_d[:, :], in_=osb[:, CH : 2 * CH])
        nc.vector.tensor_copy(out=osb[:, 2 * CH : 3 * CH], in_=pden[0])
        nc.sync.dma_start(out=o3_d[:, :], in_=osb[:, 2 * CH : 3 * CH])

    return nc


_NC = None


def kernel(query, key, value, mask, Wq, bq, Wk, bk, Wv, bv):
    global _NC, LAST_RESULT
    bf16 = ml_dtypes.bfloat16
    B = query.shape[0]
    assert B == 8

    if _NC is None:
        _NC = _build()
        _NC.finalize()

    def prepack(w):  # [768, 128] -> [p, t, n] layout [128, 768]
        return np.ascontiguousarray(
            w.reshape(HT, P, P).transpose(1, 0, 2).reshape(P, HT * P).astype(bf16)
        )

    wqk = prepack(np.concatenate([np.asarray(Wq), np.asarray(Wk)], axis=1))
    wvv = prepack(np.concatenate([np.asarray(Wv), np.asarray(Wv)], axis=1))
    ict = np.eye(P, dtype=bf16)
    bqk = np.concatenate([np.asarray(bq), np.asarray(bk)]).astype(np.float32)
    bvv = np.concatenate([np.asarray(bv), np.asarray(bv)]).astype(np.float32)
    bkk = np.concatenate([np.asarray(bk), np.asarray(bk)]).astype(np.float32)
    bqq = np.concatenate([np.asarray(bq), np.asarray(bq)]).astype(np.float32)

    def pack_kq(x):  # [2048, 768] -> [128, 4*6*512] SBUF-native
        return np.ascontiguousarray(
            np.asarray(x).reshape(NCH, CH, HT, P).transpose(3, 0, 2, 1)
            .reshape(P, NCH * HT * CH).astype(bf16)
        )

    def pack_v(x):  # wvv | [2048, 768] -> [128, 768 + 2*6*1024]
        vp = (np.asarray(x).reshape(2, 2 * CH, HT, P).transpose(3, 0, 2, 1)
              .reshape(P, 2 * HT * 2 * CH).astype(bf16))
        return np.ascontiguousarray(np.concatenate([wvv, vp], axis=1))

    in_maps = []
    for b in range(B):
        mb = ((np.asarray(mask[b], np.float32) - 1.0) * 1e9).reshape(NT, P).T
        cst = np.ascontiguousarray(
            np.concatenate(
                [bqk[:, None], bvv[:, None], bkk[:, None], bqq[:, None], mb],
                axis=1,
            )
        ).astype(np.float32)
        in_maps.append(
            {
                "qpk": pack_kq(query[b]),
                "kpk": pack_kq(key[b]),
                "vpk": pack_v(value[b]),
                "wqk": wqk,
                "ict": ict,
                "cst": cst,
            }
        )

    res = run_bass_kernel_spmd(
        _NC,
        in_maps,
        core_ids=list(range(8)),
        trace=bool(os.environ.get("KERNEL_TRACE")),
    )
    LAST_RESULT = res
    out = np.empty((B, S, D), dtype=np.float32)
    for b in range(B):
        o1 = np.asarray(res.results[b]["o1"]).astype(np.float32)  # chunks 0,1
        o2 = np.asarray(res.results[b]["o2"]).astype(np.float32)  # chunks 2,3
        o3 = np.asarray(res.results[b]["o3"]).astype(np.float32)  # denominators
        for ci in range(NCH):
            oh = o1 if ci < 2 else o2
            blk = oh[(ci % 2) * D : (ci % 2) * D + D, :]  # O^T chunk ci
            den = o3[32 * ci, :]
            out[b, ci * CH : (ci + 1) * CH, :] = (blk / den[None, :]).T
    return out


# revision 13
# speedup vs baseline: 1.1828x; 1.1828x over previous
"""Single-head attention (B=8, S=2048, H=768, D=64) on 8 TRN2 NeuronCores.

Data-parallel over batch: core b computes batch element b end to end; no
collectives. Host prepacks Q/K/V into SBUF-native [128, ...] bf16 layouts
(linear DMA, 128 big descriptors per transfer) and does the final softmax
divide + transpose; the device ships unnormalized O^T plus denominators
as raw f32 PSUM DMAs.

Schedule (v3, from perfetto analysis):
  - The ACT exp stream (32 x [128,1024], ~35.6us busy) is the pacing
    resource; everything else is arranged so exp never waits.
  - Input DMA rides the SP queue in exact consumption order
    k0,q0,q1,k1,v0,k2,k3,v1,q2,q3 (quarter granules for k/q). The
    weights ride the ACT queue: wqk first (critical), ident+cst after.
    wvv is prepended to the v half-0 transfer.
  - ~55 PE warmup matmuls on a memset tile bridge the whole DMA lead-in
    so the HAM clock gate never closes before the projections.
  - Projections are single-chunk col-packed concurrent pairs; the k/q
    chunk chain (DMA sem -> proj -> DVE drain -> score) is the first-exp
    critical path: ~19us.
  - Softmax denominators: DVE sums pth tile pairs, PE does 8 4x-col-
    packed quad matmuls on the pair sums (2.8us PE instead of 8.8).
  - O^T half-0 pairs spread ~1 per period through phase B; v half-1
    projection+transpose pulled into phase-A slack (v1 lands ~29us).
"""

import os
from contextlib import ExitStack

import numpy as np
import ml_dtypes

import concourse.bass as bass
import concourse.mybir as mybir
import concourse.tile as tile
from concourse import bacc
from concourse.bass_utils import run_bass_kernel_spmd

S, H, D = 2048, 768, 64
P = 128
NT = S // P      # 16 sk tiles
HT = H // P      # 6 h tiles
CH = 512         # sq chunk = matmul free dim = PSUM bank
NCH = S // CH    # 4
BF = mybir.dt.bfloat16
F32 = mybir.dt.float32
AF = mybir.ActivationFunctionType

LAST_RESULT = None  # BassKernelResults of the most recent run (for test.py)


def _build(debug=False):
    nc = bacc.Bacc()
    qpk_d = nc.declare_dram_parameter("qpk", [P, NCH * HT * CH], BF, isOutput=False)
    kpk_d = nc.declare_dram_parameter("kpk", [P, NCH * HT * CH], BF, isOutput=False)
    # v prepack: {wvv | v half0 | v half1}
    VW = HT * 2 * CH  # 6144 elements per v half
    vpk_d = nc.declare_dram_parameter("vpk", [P, HT * P + 2 * VW], BF, isOutput=False)
    wqk_d = nc.declare_dram_parameter("wqk", [P, HT * P], BF, isOutput=False)
    ict_d = nc.declare_dram_parameter("ict", [P, P], BF, isOutput=False)
    cst_d = nc.declare_dram_parameter("cst", [P, 4 + NT], F32, isOutput=False)
    o1_d = nc.declare_dram_parameter("o1", [P, CH], BF, isOutput=True)
    o2_d = nc.declare_dram_parameter("o2", [P, CH], BF, isOutput=True)
    o3_d = nc.declare_dram_parameter("o3", [P, CH], BF, isOutput=True)

    with ExitStack() as ctx:
        tc = ctx.enter_context(tile.TileContext(nc))
        consts = ctx.enter_context(tc.tile_pool(name="consts", bufs=1))
        stages = ctx.enter_context(tc.tile_pool(name="stages", bufs=1))
        persist = ctx.enter_context(tc.tile_pool(name="persist", bufs=1))
        ppool = ctx.enter_context(tc.tile_pool(name="ppool", bufs=2 * NT))
        psc = ctx.enter_context(tc.tile_pool(name="psc", bufs=1, space="PSUM"))
        psw = ctx.enter_context(tc.tile_pool(name="psw", bufs=1, space="PSUM"))
        psav = ctx.enter_context(tc.tile_pool(name="psav", bufs=1, space="PSUM"))

        # ---- constants on the scalar queue: wqk first (critical path) ----
        wqk_sb = consts.tile([P, HT * P], BF, tag="wqk")
        nc.scalar.dma_start(out=wqk_sb, in_=wqk_d[:, :])
        ict_sb = consts.tile([P, P], BF, tag="ict")
        nc.scalar.dma_start(out=ict_sb, in_=ict_d[:, :])
        cst_sb = consts.tile([P, 4 + NT], F32, tag="cst")
        nc.scalar.dma_start(out=cst_sb, in_=cst_d[:, :])
        ident_bf = ict_sb
        bvv_sb = cst_sb[:, 1:2]
        bkk_sb = cst_sb[:, 2:3]
        bqq_sb = cst_sb[:, 3:4]
        mb_sb = cst_sb[:, 4 : 4 + NT]
        warmT = consts.tile([P, P], BF, tag="warmT")
        nc.vector.memset(warmT, 0.0)
        ones32 = consts.tile([P, 32], BF, tag="ones32")
        nc.vector.memset(ones32, 1.0)

        def w_qk(h, lo, hi):  # packed q|k weight slice [128, hi-lo]
            return wqk_sb[:, h * P + lo : h * P + hi]

        # ---- input staging: linear transfers on the sync queue in exact
        # consumption order k0,q0,q1,k1,v0(+wvv),k2,k3,v1,q2,q3 ----
        stq = stages.tile([P, NCH, HT * CH], BF, tag="stq")
        stk = stages.tile([P, NCH, HT * CH], BF, tag="stk")
        stv = stages.tile([P, 2, VW], BF, tag="stv")
        wvv_sb = stages.tile([P, HT * P], BF, tag="wvv")
        QW = HT * CH  # 3072 elements per k/q quarter

        def dma_kq(dst, src, c):
            nc.sync.dma_start(out=dst[:, c, :], in_=src[:, c * QW : (c + 1) * QW])

        dma_kq(stk, kpk_d, 0)
        dma_kq(stq, qpk_d, 0)
        dma_kq(stq, qpk_d, 1)
        dma_kq(stk, kpk_d, 1)
        dma_kq(stk, kpk_d, 2)
        dma_kq(stk, kpk_d, 3)
        nc.sync.dma_start(out=wvv_sb, in_=vpk_d[:, 0 : HT * P])
        nc.sync.dma_start(out=stv[:, 0, :], in_=vpk_d[:, HT * P : HT * P + VW])
        nc.sync.dma_start(out=stv[:, 1, :], in_=vpk_d[:, HT * P + VW : HT * P + 2 * VW])
        dma_kq(stq, qpk_d, 2)
        dma_kq(stq, qpk_d, 3)

        def w_vv(h, lo, hi):
            return wvv_sb[:, h * P + lo : h * P + hi]

        def st_kq(st, h, c):  # [128, 512] h-tile slice of chunk c
            return st[:, c, h * CH : (h + 1) * CH]

        def st_v(h, half, i):  # [128, 512] h-tile slice of chunk 2*half+i
            return stv[:, half, h * 2 * CH + i * CH : h * 2 * CH + (i + 1) * CH]

        # ---- persistent SBUF tensors ----
        qqT_sb = persist.tile([P, S], BF, tag="qqT")
        kkT_sb = persist.tile([P, S], BF, tag="kkT")
        vT2_sb = persist.tile([P, S // 2], BF, tag="vT2")
        vE_sb = persist.tile([P, NT * D], BF, tag="vE")
        osb = persist.tile([P, 3 * CH], BF, tag="osb")  # output staging

        # ---- PE warmup: bridge the whole DMA lead-in ----
        warm = psw.tile([P, CH], F32, tag="aux", name="warm")
        for i in range(55):
            nc.tensor.matmul(
                warm[:, :P],
                lhsT=warmT,
                rhs=warmT,
                start=True,
                stop=True,
                skip_group_check=True,
            )

        # ---- helper blocks ----
        def kq1proj(kind, c, ptag):
            """single-chunk projection (kind 0 = k -> kkT, 1 = q -> qqT):
            one col-packed concurrent pair produces native+duplicated
            partition halves of chunk c in 6x512 cycles, then one [128,512]
            bias drain."""
            dst = kkT_sb if kind == 0 else qqT_sb
            bias = bkk_sb if kind == 0 else bqq_sb
            wlo = D if kind == 0 else 0
            stx = stk if kind == 0 else stq
            pp = psav.tile([P, CH], F32, tag=ptag, name=f"pp{kind}_{c}")
            for h in range(HT):
                nc.tensor.matmul(
                    pp[:D, :],
                    lhsT=w_qk(h, wlo, wlo + D),
                    rhs=st_kq(stx, h, c),
                    start=(h == 0),
                    stop=(h == HT - 1),
                    tile_position=(0, 0),
                    skip_group_check=True,
                )
                nc.tensor.matmul(
                    pp[D:, :],
                    lhsT=w_qk(h, wlo, wlo + D),
                    rhs=st_kq(stx, h, c),
                    start=(h == 0),
                    stop=(h == HT - 1),
                    tile_position=(0, D),
                    skip_group_check=True,
                )
            nc.vector.tensor_scalar_add(
                out=dst[:, c * CH : (c + 1) * CH], in0=pp, scalar1=bias
            )

        pth = {}

        def scores_exp(t, half):
            """scores for sk-tile t over sq half (row group alternates with
            t to hide LDWEIGHTS), exp straight into a bf16 tile."""
            lo, hi = (0, D) if t % 2 == 0 else (D, P)
            ps = psc.tile([P, 2 * CH], F32, tag=f"sc{t % 2}",
                          name=f"ps{t}_{half}")
            for sub in range(2):
                c = 2 * half + sub
                nc.tensor.matmul(
                    ps[:, sub * CH : (sub + 1) * CH],
                    lhsT=kkT_sb[lo:hi, t * P : (t + 1) * P],
                    rhs=qqT_sb[lo:hi, c * CH : (c + 1) * CH],
                    start=True,
                    stop=True,
                    tile_position=(lo, 0),
                    skip_group_check=True,
                )
            pt = ppool.tile([P, 2 * CH], BF, tag="pT", name=f"pt{t}_{half}")
            nc.scalar.activation(
                out=pt,
                in_=ps,
                func=AF.Exp,
                bias=mb_sb[:, t : t + 1],
                scale=0.125,
            )
            pth[t, half] = pt

        pav = {}

        def vE_slice(t):
            b = (t // 8) * 4 + (t % 4)
            half = (t % 8) // 4
            return vE_sb[:, b * P + half * D : b * P + half * D + D]

        def av(t, half):
            """col-packed concurrent O^T accumulation pair for sk-tile t:
            chunk 2*half -> partitions 0:64, chunk 2*half+1 -> 64:128."""
            key = "av23" if half == 0 else "av01"
            if half not in pav:
                pav[half] = psav.tile([P, CH], F32, tag=key, name=f"pav{half}")
            vt = vE_slice(t)
            nc.tensor.matmul(
                pav[half][:D, :],
                lhsT=vt,
                rhs=pth[t, half][:, :CH],
                start=(t == 0),
                stop=(t == NT - 1),
                tile_position=(0, 0),
                skip_group_check=True,
            )
            nc.tensor.matmul(
                pav[half][D:, :],
                lhsT=vt,
                rhs=pth[t, half][:, CH:],
                start=(t == 0),
                stop=(t == NT - 1),
                tile_position=(0, D),
                skip_group_check=True,
            )

        pden = [None]

        def den4(t):
            """4x col-packed concurrent denominator matmuls: chunk ci's
            softmax denominator accumulates in partitions 32ci:32ci+32."""
            if pden[0] is None:
                pden[0] = psw.tile([P, CH], F32, tag="den", name="pden")
            for ci in range(NCH):
                nc.tensor.matmul(
                    pden[0][32 * ci : 32 * (ci + 1), :],
                    lhsT=ones32[:, :],
                    rhs=pth[t, ci // 2][:, (ci % 2) * CH : (ci % 2 + 1) * CH],
                    start=(t == 0),
                    stop=(t == NT - 1),
                    tile_position=(0, 32 * ci),
                    skip_group_check=True,
                )

        pv_t = {}

        def v_proj_mm(u, hs=(0, HT)):
            """v projection matmuls (h range hs) + bias for chunk-pair u:
            chunk 2u -> partitions 0:64, chunk 2u+1 -> 64:128 (stacked)."""
            if u not in pv_t:
                pv_t[u] = psw.tile([P, CH], F32,
                                   tag="den" if u == 0 else "aux",
                                   name=f"pv{u}")
            pv = pv_t[u]
            for h in range(*hs):
                nc.tensor.matmul(
                    pv[:D, :],
                    lhsT=w_vv(h, 0, D),
                    rhs=st_v(h, u, 0),
                    start=(h == 0),
                    stop=(h == HT - 1),
                    tile_position=(0, 0),
                    skip_group_check=True,
                )
                nc.tensor.matmul(
                    pv[D:, :],
                    lhsT=w_vv(h, D, P),
                    rhs=st_v(h, u, 1),
                    start=(h == 0),
                    stop=(h == HT - 1),
                    tile_position=(0, D),
                    skip_group_check=True,
                )
            if hs[1] == HT:
                nc.vector.tensor_scalar_add(
                    out=vT2_sb[:, u * CH : (u + 1) * CH], in0=pv,
                    scalar1=bvv_sb,
                )

        def v_transpose(u, j):
            """one [128,128] PE-transpose block of vT2 into vE
            (block b = 4u+j holds sk tiles 8u+j and 8u+j+4)."""
            pt = psw.tile([P, P], BF, tag="den" if u == 0 else "aux",
                          name=f"ptv{u}_{j}")
            nc.tensor.transpose(
                pt,
                in_=vT2_sb[:, u * CH + j * P : u * CH + (j + 1) * P],
                identity=ident_bf,
            )
            b = 4 * u + j
            nc.vector.tensor_copy(out=vE_sb[:, b * P : (b + 1) * P], in_=pt)

        # ---- phase A schedule ----
        kq1proj(0, 0, "av01")   # k chunk 0 (arrives first)
        kq1proj(1, 0, "av23")   # q chunk 0
        kq1proj(1, 1, "av01")   # q chunk 1
        scores_exp(0, 0)
        scores_exp(1, 0)
        scores_exp(2, 0)
        kq1proj(0, 1, "av23")   # k chunk 1
        scores_exp(3, 0)
        scores_exp(4, 0)
        scores_exp(5, 0)
        kq1proj(0, 2, "av01")   # k chunk 2
        scores_exp(6, 0)
        scores_exp(7, 0)
        scores_exp(8, 0)
        kq1proj(0, 3, "av23")   # k chunk 3
        scores_exp(9, 0)
        scores_exp(10, 0)
        v_proj_mm(0)            # v half 0 (lands ~25us)
        scores_exp(11, 0)
        v_transpose(0, 0)
        v_transpose(0, 1)
        scores_exp(12, 0)
        v_transpose(0, 2)
        v_transpose(0, 3)
        scores_exp(13, 0)
        v_proj_mm(1, (0, 3))    # v half 1 (lands ~29us)
        scores_exp(14, 0)
        v_proj_mm(1, (3, HT))
        scores_exp(15, 0)
        kq1proj(1, 2, "av01")   # q chunk 2
        kq1proj(1, 3, "av23")   # q chunk 3

        # ---- phase B: exp stream; av1+den trail 2 tiles; av0 spread ~1
        # per period; v half-1 transposes at the start ----
        scores_exp(0, 1)
        v_transpose(1, 0)
        v_transpose(1, 1)
        scores_exp(1, 1)
        v_transpose(1, 2)
        v_transpose(1, 3)
        av(0, 0)
        scores_exp(2, 1)
        av(0, 1)
        den4(0)
        av(1, 0)
        scores_exp(3, 1)
        av(1, 1)
        den4(1)
        av(2, 0)
        scores_exp(4, 1)
        av(2, 1)
        den4(2)
        av(3, 0)
        scores_exp(5, 1)
        av(3, 1)
        den4(3)
        av(4, 0)
        scores_exp(6, 1)
        av(4, 1)
        den4(4)
        av(5, 0)
        scores_exp(7, 1)
        av(5, 1)
        den4(5)
        av(6, 0)
        scores_exp(8, 1)
        av(6, 1)
        den4(6)
        av(7, 0)
        scores_exp(9, 1)
        av(7, 1)
        den4(7)
        av(8, 0)
        av(9, 0)
        scores_exp(10, 1)
        av(8, 1)
        den4(8)
        av(10, 0)
        av(11, 0)
        scores_exp(11, 1)
        av(9, 1)
        den4(9)
        av(12, 0)
        av(13, 0)
        scores_exp(12, 1)
        av(10, 1)
        den4(10)
        av(14, 0)
        av(15, 0)
        nc.vector.tensor_copy(out=osb[:, 0:CH], in_=pav[0])
        nc.sync.dma_start(out=o1_d[:, :], in_=osb[:, 0:CH])
        scores_exp(13, 1)
        av(11, 1)
        den4(11)
        scores_exp(14, 1)
        av(12, 1)
        den4(12)
        scores_exp(15, 1)
        av(13, 1)
        den4(13)
        av(14, 1)
        den4(14)
        av(15, 1)
        den4(15)

        # ---- epilogue ----
        nc.vector.tensor_copy(out=osb[:, CH : 2 * CH], in_=pav[1])
        nc.sync.dma_start(out=o2_d[:, :], in_=osb[:, CH : 2 * CH])
        nc.vector.tensor_copy(out=osb[:, 2 * CH : 3 * CH], in_=pden[0])
        nc.sync.dma_start(out=o3_d[:, :], in_=osb[:, 2 * CH : 3 * CH])

    return nc


_NC = None


def kernel(query, key, value, mask, Wq, bq, Wk, bk, Wv, bv):
    global _NC, LAST_RESULT
    bf16 = ml_dtypes.bfloat16
    B = query.shape[0]
    assert B == 8

    if _NC is None:
        _NC = _build()
        _NC.finalize()

    def prepack(w):  # [768, 128] -> [p, t, n] layout [128, 768]
        return np.ascontiguousarray(
            w.reshape(HT, P, P).transpose(1, 0, 2).reshape(P, HT * P).astype(bf16)
        )

    wqk = prepack(np.concatenate([np.asarray(Wq), np.asarray(Wk)], axis=1))
    wvv = prepack(np.concatenate([np.asarray(Wv), np.asarray(Wv)], axis=1))
    ict = np.eye(P, dtype=bf16)
    bqk = np.concatenate([np.asarray(bq), np.asarray(bk)]).astype(np.float32)
    bvv = np.concatenate([np.asarray(bv), np.asarray(bv)]).astype(np.float32)
    bkk = np.concatenate([np.asarray(bk), np.asarray(bk)]).astype(np.float32)
    bqq = np.concatenate([np.asarray(bq), np.asarray(bq)]).astype(np.float32)

    def pack_kq(x):  # [2048, 768] -> [128, 4*6*512] SBUF-native
        return np.ascontiguousarray(
            np.asarray(x).reshape(NCH, CH, HT, P).transpose(3, 0, 2, 1)
            .reshape(P, NCH * HT * CH).astype(bf16)
        )

    def pack_v(x):  # wvv | [2048, 768] -> [128, 768 + 2*6*1024]
        vp = (np.asarray(x).reshape(2, 2 * CH, HT, P).transpose(3, 0, 2, 1)
              .reshape(P, 2 * HT * 2 * CH).astype(bf16))
        return np.ascontiguousarray(np.concatenate([wvv, vp], axis=1))

    in_maps = []
    for b in range(B):
        mb = ((np.asarray(mask[b], np.float32) - 1.0) * 1e9).reshape(NT, P).T
        cst = np.ascontiguousarray(
            np.concatenate(
                [bqk[:, None], bvv[:, None], bkk[:, None], bqq[:, None], mb],
                axis=1,
            )
        ).astype(np.float32)
        in_maps.append(
            {
                "qpk": pack_kq(query[b]),
                "kpk": pack_kq(key[b]),
                "vpk": pack_v(value[b]),
                "wqk": wqk,
                "ict": ict,
                "cst": cst,
            }
        )

    res = run_bass_kernel_spmd(
        _NC,
        in_maps,
        core_ids=list(range(8)),
        trace=bool(os.environ.get("KERNEL_TRACE")),
    )
    LAST_RESULT = res
    out = np.empty((B, S, D), dtype=np.float32)
    for b in range(B):
        o1 = np.asarray(res.results[b]["o1"]).astype(np.float32)  # chunks 0,1
        o2 = np.asarray(res.results[b]["o2"]).astype(np.float32)  # chunks 2,3
        o3 = np.asarray(res.results[b]["o3"]).astype(np.float32)  # denominators
        for ci in range(NCH):
            oh = o1 if ci < 2 else o2
            blk = oh[(ci % 2) * D : (ci % 2) * D + D, :]  # O^T chunk ci
            den = o3[32 * ci, :]
            out[b, ci * CH : (ci + 1) * CH, :] = (blk / den[None, :]).T
    return out
